# revision 23
# baseline (speedup 1.0000x reference)
"""Trainium2 Bass kernel for nn_ADNN_57501022159565 (GNN message passing).

Reference computation (B=4096, D=256, N=128, F=128, O=256):
    conn = softmax(attention, axis=1)                      # [N, N]
    W_s  = node_weight[:, :F, :]                           # [N, F, F]
    W_x  = node_weight[:, F:, :]                           # [N, D, F]
    X    = einsum('bd,ndf->nbf', x, W_x) + bias[:,None,:]  # [N, B, F]
    S_1  = relu(X)                                         # (states0 = 0)
    S_k  = relu(einsum('ij,jbf->ibf', conn, S_{k-1}) @ W_s + X)
    out  = mean_n(S_T) @ out_w + out_b                     # [B, O]

Data-parallel over batch: B=4096 split across 8 NeuronCores (512 rows
each), weights replicated, no collectives. bf16 compute, f32 PSUM.

v3 design vs the 1.43ms v2 baseline:
  - The feature-major -> node-major rearrange of the state no longer
    issues 128 per-node HWDGE DMAs per chunk (the HWDGE ring was 59%
    busy, serializing the whole kernel). Instead the staging tiles are
    scattered into per-(iteration, chunk, half) DRAM scratch tensors in
    node-major layout (one HWDGE DMA per staging tile per half) and read
    back densely (one HWDGE DMA per half).
  - States are double-half-buffered ([N, F, 64] tiles, bufs=3) so the
    next chunk's state read overlaps the current chunk's aggregation.
  - Phase 1's node_bias rides the PSUM evictions (ACT Identity bias /
    DVE tensor_scalar_add with a per-partition bias column) instead of
    identity-matmul accumulation.
  - The iteration X-add still rides PSUM accumulation (identity MM).
  - Cross-chunk work (next chunk's S_1 emission, pair-1 phase 1) is
    emitted between k=2 and k=3 so it covers the rearrange gap.
"""
import sys

if "/opt/trn_rl_repo" not in sys.path:
    sys.path.insert(0, "/opt/trn_rl_repo")

import numpy as np

import concourse.bass as bass
import concourse.tile as tile
from concourse import bacc, mybir
from concourse.bass import AP
from concourse.bass_utils import run_bass_kernel_spmd
from concourse.masks import make_identity

F32 = mybir.dt.float32
BF16 = mybir.dt.bfloat16
AF = mybir.ActivationFunctionType


def _bcast(ap, n):
    """Append a stride-0 dim of size n to an AP (broadcast over batch)."""
    return AP(ap.tensor, ap.offset, list(ap.ap) + [[0, n]])

B, D, N, F, O = 4096, 256, 128, 128, 256
NCORES = 8
BLOC = B // NCORES          # 512 batch rows per core
BS = 128                    # batch chunk
HS = BS // 2                # state half-chunk
NCHUNK = BLOC // BS         # 4
NG = 16                     # node group for staging scatter-writes


def build_nc(num_iterations: int, reps: int = 1):
    T = int(num_iterations)
    nc = bacc.Bacc(None, target_bir_lowering=False)
    x_ext = nc.declare_dram_parameter("x", [BLOC, D], F32, isOutput=False)
    nw_ext = nc.declare_dram_parameter("node_weight", [N, F + D, F], F32, isOutput=False)
    nb_ext = nc.declare_dram_parameter("node_bias", [N, F], F32, isOutput=False)
    att_ext = nc.declare_dram_parameter("attention", [N, N], F32, isOutput=False)
    ow_ext = nc.declare_dram_parameter("out_w", [F, O], F32, isOutput=False)
    ob_ext = nc.declare_dram_parameter("out_b", [1, O], F32, isOutput=False)
    out_ext = nc.declare_dram_parameter("out", [BLOC, O], F32, isOutput=True)

    nw_r = nw_ext[:].rearrange("n d g -> d n g")  # (D+F, N, F) strided view

    # DRAM scratch for the state rearrange: one tensor per (rep, emission,
    # chunk) so there are no WAR/WAW hazards. Feature-major [F, N, BS]
    # (same layout as the staging tiles -> dense writes; the reads do the
    # node-major scatter, which HBM tolerates better than scattered
    # writes).
    n_emit = max(0, T - 1)
    sd = {}
    with tile.TileContext(nc) as tc:
        for r in range(reps):
            for k in range(n_emit):
                for c in range(NCHUNK):
                    sd[(r, k, c)] = nc.dram_tensor(
                        f"sd_{r}_{k}_{c}", [F, N, BS], BF16)
        with (
            tc.tile_pool(name="const", bufs=1) as cp,
            tc.tile_pool(name="wp", bufs=1) as wp,
            tc.tile_pool(name="big", bufs=1) as bp,
            tc.tile_pool(name="tmp", bufs=1) as tp,
            tc.tile_pool(name="stage", bufs=2) as sp,
            tc.tile_pool(name="wxp", bufs=2) as wxp,
            tc.tile_pool(name="ps_a", bufs=3, space="PSUM") as ps_a,
            tc.tile_pool(name="ps_m1", bufs=2, space="PSUM") as ps_m1,
            tc.tile_pool(name="ps_mI", bufs=2, space="PSUM") as ps_mI,
            tc.tile_pool(name="ps_o", bufs=1, space="PSUM") as ps_o,
        ):
            for _rep in range(reps):
                _body(nc, tc, T, cp, wp, bp, tp, sp, wxp, ps_a, ps_m1, ps_mI,
                      ps_o, x_ext, nw_r, nb_ext, att_ext, ow_ext, ob_ext,
                      out_ext,
                      {(k, c): sd[(_rep, k, c)]
                       for k in range(n_emit) for c in range(NCHUNK)})
    nc.compile()
    return nc


class _Alt:
    """Alternate eviction work between DVE and ACT."""

    def __init__(self, nc):
        self.nc = nc
        self.i = 0

    def copy(self, dst, src):
        self.i += 1
        if self.i % 2:
            self.nc.vector.tensor_copy(dst, src)
        else:
            self.nc.scalar.activation(dst, src, AF.Copy)

    def relu(self, dst, src):
        self.i += 1
        if self.i % 2:
            self.nc.scalar.activation(dst, src, AF.Relu)
        else:
            self.nc.vector.tensor_scalar_max(dst, src, 0.0)

    def add_bias(self, dst, src, bias_col):
        """dst = src + bias (bias: per-partition column AP)."""
        self.i += 1
        if self.i % 2:
            self.nc.scalar.activation(dst, src, AF.Identity, bias=bias_col)
        else:
            self.nc.vector.tensor_scalar_add(dst, src, bias_col)


def _body(nc, tc, T, cp, wp, bp, tp, sp, wxp, ps_a, ps_m1, ps_mI, ps_o,
          x_ext, nw_r, nb_ext, att_ext, ow_ext, ob_ext, out_ext, sd):
    alt = _Alt(nc)
    # ---------------- setup: constants ----------------
    ident = cp.tile([128, 128], BF16, tag="ident")
    make_identity(nc, ident[:])
    identf = cp.tile([128, 128], F32, tag="identf")
    make_identity(nc, identf[:])
    ones = cp.tile([1, 128], BF16, tag="ones")
    nc.gpsimd.memset(ones[:], 1.0)

    # x -> xT (d | b) bf16, two d-halves (f32 HWDGE load, PE transpose)
    xT = [cp.tile([128, BLOC], BF16, tag=f"xT{dc}", name=f"xT{dc}") for dc in range(2)]
    for c in range(NCHUNK):
        xs = sp.tile([BS, D], F32, tag="xs", bufs=2, name="xs")
        nc.sync.dma_start(xs[:], x_ext[c * BS:(c + 1) * BS, :])
        for dc in range(2):
            px = ps_o.tile([128, BS], F32, tag="poo")
            nc.tensor.transpose(px[:], xs[:, dc * 128:(dc + 1) * 128], identf[:])
            nc.vector.tensor_copy(xT[dc][:, c * BS:(c + 1) * BS], px[:])

    # out_w scaled by 1/N (folds the node-mean), out_b
    ow = cp.tile([F, O], BF16, tag="ow")
    nc.gpsimd.dma_start(ow[:], ow_ext[:])  # cast f32->bf16
    nc.vector.tensor_scalar_mul(ow[:], ow[:], 1.0 / N)
    ob = cp.tile([1, O], BF16, tag="ob")
    nc.gpsimd.dma_start(ob[:], ob_ext[:])  # cast f32->bf16

    if T == 0:
        po = ps_o.tile([BS, O], F32, tag="poo")
        nc.tensor.matmul(po[:], ones[:, 0:BS], ob[:], start=True, stop=True)
        ot = sp.tile([BS, O], F32, tag="ot", bufs=2, name="ot")
        nc.vector.tensor_copy(ot[:], po[:])
        for c in range(NCHUNK):
            nc.sync.dma_start(out_ext[c * BS:(c + 1) * BS, :], ot[:])
        return

    # softmax(attention) -> conn (bf16), then connT via PE transpose
    att = tp.tile([N, N], F32, tag="att")
    nc.sync.dma_start(att[:], att_ext[:])
    mx = tp.tile([N, 1], F32, tag="mx")
    nc.vector.tensor_reduce(mx[:], att[:], axis=mybir.AxisListType.X,
                            op=mybir.AluOpType.max, negate=True)
    ex = tp.tile([N, N], F32, tag="ex")
    nc.scalar.activation(ex[:], att[:], AF.Exp, bias=mx[:, 0:1])
    sm = tp.tile([N, 1], F32, tag="sm")
    nc.vector.tensor_reduce(sm[:], ex[:], axis=mybir.AxisListType.X,
                            op=mybir.AluOpType.add)
    rc = tp.tile([N, 1], F32, tag="rc")
    nc.vector.reciprocal(rc[:], sm[:])
    conn = tp.tile([N, N], BF16, tag="conn")
    nc.vector.tensor_scalar_mul(conn[:], ex[:], rc[:, 0:1])
    pt = ps_o.tile([N, N], BF16, tag="poo")
    nc.tensor.transpose(pt[:], conn[:], ident[:])
    connT = cp.tile([N, N], BF16, tag="connT")
    nc.vector.tensor_copy(connT[:], pt[:])

    # node_bias -> bias_fm (f | n) bf16 (folded via identity-MM into PSUM)
    nb_bf = tp.tile([N, F], BF16, tag="nb_bf")
    nc.gpsimd.dma_start(nb_bf[:], nb_ext[:])
    pb = ps_o.tile([F, N], BF16, tag="poo")
    nc.tensor.transpose(pb[:], nb_bf[:], ident[:])
    bias_fm = cp.tile([F, N], BF16, tag="bias_fm")
    nc.vector.tensor_copy(bias_fm[:], pb[:])

    # W_s resident (f | n, g) bf16 via SWDGE cast-DMA
    W_s = wp.tile([F, N, F], BF16, tag="W_s")

    X_tiles = {}     # chunk -> X tile

    NGX = 8          # node group for streamed W_x tiles

    def phase1_pair(p):
        """xW for chunks (2p, 2p+1): free dim 256, W_x streamed through a
        small rotating pool (SWDGE cast-DMA f32->bf16 on the Pool engine).
        node_bias is folded into the evictions (per-partition bias col)."""
        c0 = 2 * p
        cb = c0 * BS
        for c in (c0, c0 + 1):
            X_tiles[c] = bp.tile([F, N, BS], BF16, tag="X", bufs=2, name=f"X{c}")
        for base in range(0, N, NGX):
            wx0 = wxp.tile([128, NGX, F], BF16, tag="wx0", name="wx0")
            wx1 = wxp.tile([128, NGX, F], BF16, tag="wx1", name="wx1")
            nc.gpsimd.dma_start(wx0[:], nw_r[F:F + 128, base:base + NGX, :])
            nc.gpsimd.dma_start(wx1[:], nw_r[F + 128:F + 256, base:base + NGX, :])
            for n2 in range(0, NGX, 2):
                n0 = base + n2
                pm = ps_m1.tile([F, 2, 256], F32, tag="pm1")
                # one PSUM group per bank: bias broadcast writes first
                # (start=True overwrites), then the W_x matmuls accumulate
                nc.tensor.matmul(pm[:], ident[:], _bcast(bias_fm[:, n0:n0 + 2], 256),
                                 start=True, stop=False)
                for q in range(2):
                    nc.tensor.matmul(pm[:, q, :], wx0[:, n2 + q, :],
                                     xT[0][:, cb:cb + 256], start=False,
                                     stop=False)
                    nc.tensor.matmul(pm[:, q, :], wx1[:, n2 + q, :],
                                     xT[1][:, cb:cb + 256], start=False,
                                     stop=(q == 1))
                for ci in range(2):
                    alt.copy(X_tiles[c0 + ci][:, n0:n0 + 2, :],
                             pm[:, :, ci * BS:(ci + 1) * BS])

    def alloc_state(k, c):
        """Two batch-half state tiles [N, F, HS] from a 3-deep ring."""
        return [bp.tile([N, F, HS], BF16, tag="S", bufs=3,
                        name=f"S{k}_{c}_{h}") for h in range(2)]

    def emit_group(c, k, g0, gt, Sh):
        """One node group: dense write of the staging tile to DRAM, then
        two node-major scatter-reads (one per batch half) into Sh."""
        sdt = sd[(k, c)]
        nc.sync.dma_start(sdt[:, g0:g0 + NG, :], gt[:])
        for h in range(2):
            nc.sync.dma_start(
                Sh[h][g0:g0 + NG, :, :],
                sdt[:, g0:g0 + NG, h * HS:(h + 1) * HS].rearrange(
                    "f n b -> n f b"))

    def s1_emit(c):
        """S_1 = relu(X) -> staging -> DRAM -> node-major halves."""
        X = X_tiles[c]
        Sh = alloc_state(0, c)
        for g0 in range(0, N, NG):
            gt = sp.tile([F, NG, BS], BF16, tag="gt", bufs=3, name="gt")
            alt.relu(gt[:], X[:, g0:g0 + NG, :])
            emit_group(c, 0, g0, gt, Sh)
        return Sh

    def stationary(Sh, b):
        t = Sh[b // HS]
        return t[:, :, b % HS]

    def iteration(c, k, Sh):
        """One iteration k for chunk c: aggregation + per-node transform.
        Returns the next state half-tiles, or None for the last iteration
        (relu written into X in place)."""
        last = (k == T)
        X = X_tiles[c]
        agg = bp.tile([F, N, BS], BF16, tag="agg", name="agg")
        for b0 in range(0, BS, 4):
            pa = ps_a.tile([F, 4, N], F32, tag="pa")
            for q in range(4):
                nc.tensor.matmul(pa[:, q, :], stationary(Sh, b0 + q), connT[:],
                                 start=True, stop=True)
            # strided eviction (free at 1x) -> contiguous transform rhs
            alt.copy(agg[:, :, b0:b0 + 4].rearrange("f n b -> f b n"), pa[:])
        S_next = None if last else alloc_state(k - 1, c)
        gt = None
        for i0 in range(0, N, 4):
            pm = ps_mI.tile([F, 4, BS], F32, tag="pmI")
            # X written first (start=True overwrites the bank), then the
            # per-node transforms accumulate into their slices
            nc.tensor.matmul(pm[:], ident[:], X[:, i0:i0 + 4, :],
                             start=True, stop=False)
            for q in range(4):
                nc.tensor.matmul(pm[:, q, :], W_s[:, i0 + q, :],
                                 agg[:, i0 + q, :], start=False,
                                 stop=(q == 3))
            if last:
                alt.relu(X[:, i0:i0 + 4, :], pm[:])
            else:
                if i0 % NG == 0:
                    gt = sp.tile([F, NG, BS], BF16, tag="gt", bufs=3,
                                 name="gt")
                alt.relu(gt[:, (i0 % NG):(i0 % NG) + 4, :], pm[:])
                if i0 % NG == NG - 4:
                    emit_group(c, k - 1, i0 - NG + 4, gt, S_next)
        return S_next

    def s1_to_slast(c):
        """T==1: S_T = relu(X) in place."""
        X = X_tiles[c]
        nc.vector.tensor_scalar_max(X[:], X[:], 0.0)
        return X

    def mean_and_out(c, ST):
        """Node-mean of the final state ST [f, n, b] by in-place halving
        adds over the node dim, then a strided reduce + final linear."""
        cs = slice(c * BS, (c + 1) * BS)
        # level-1 halving into the (dead) agg slot so ST=X frees right away
        m64 = bp.tile([F, 64, BS], BF16, tag="agg", name="m64")
        nc.vector.tensor_add(m64[:], ST[:, 0:64, :], ST[:, 64:128, :])
        nc.vector.tensor_add(m64[:, 0:32, :], m64[:, 0:32, :], m64[:, 32:64, :])
        nc.vector.tensor_add(m64[:, 0:16, :], m64[:, 0:16, :], m64[:, 16:32, :])
        macc = sp.tile([F, BS], F32, tag="macc", bufs=1, name="macc")
        nc.vector.tensor_reduce(macc[:], m64[:, 0:16, :].rearrange("f n b -> f b n"),
                                axis=mybir.AxisListType.X, op=mybir.AluOpType.add)
        mean_bf = sp.tile([F, BS], BF16, tag="mean_bf", bufs=1)
        nc.vector.tensor_copy(mean_bf[:], macc[:])
        po = ps_o.tile([BS, O], F32, tag="poo")
        nc.tensor.matmul(po[:], mean_bf[:], ow[:], start=True, stop=False)
        nc.tensor.matmul(po[:], ones[:, 0:BS], ob[:], start=False, stop=True)
        ot = sp.tile([BS, O], F32, tag="ot", bufs=1, name="ot")
        nc.scalar.activation(ot[:], po[:], AF.Copy)
        nc.sync.dma_start(out_ext[cs, :], ot[:])

    # ---------------- main pipeline ----------------
    phase1_pair(0)
    # W_s loads (SWDGE, after pair-0's W_x on the Pool queue)
    for base in range(0, N, 8):
        nc.gpsimd.dma_start(W_s[:, base:base + 8, :], nw_r[0:F, base:base + 8, :])

    if T >= 2:
        S_next = s1_emit(0)
        for c in range(NCHUNK):
            Sh = S_next
            for k in range(2, T + 1):
                Sh_new = iteration(c, k, Sh)
                if k == 2:
                    # cover the k=2 -> k=3 rearrange gap with independent
                    # cross-chunk work
                    if c == 1:
                        phase1_pair(1)
                    if c + 1 < NCHUNK:
                        S_next = s1_emit(c + 1)
                Sh = Sh_new
            mean_and_out(c, X_tiles[c])
    else:
        for c in range(NCHUNK):
            if c == 2:
                phase1_pair(1)
            mean_and_out(c, s1_to_slast(c))


_NC_CACHE = {}


def _get_nc(T: int):
    if T not in _NC_CACHE:
        _NC_CACHE[T] = build_nc(T)
    return _NC_CACHE[T]


def kernel(**inputs) -> np.ndarray:
    x = np.ascontiguousarray(np.asarray(inputs["x"], dtype=np.float32))
    nw = np.ascontiguousarray(np.asarray(inputs["node_weight"], dtype=np.float32))
    nb = np.ascontiguousarray(np.asarray(inputs["node_bias"], dtype=np.float32))
    att = np.ascontiguousarray(np.asarray(inputs["attention"], dtype=np.float32))
    ow = np.ascontiguousarray(np.asarray(inputs["out_w"], dtype=np.float32))
    ob = np.ascontiguousarray(np.asarray(inputs["out_b"], dtype=np.float32)).reshape(1, O)
    T = int(np.asarray(inputs["num_iterations"]))

    nc = _get_nc(T)
    in_maps = []
    for core in range(NCORES):
        shard = x[core * BLOC:(core + 1) * BLOC]
        in_maps.append({
            "x": shard,
            "node_weight": nw,
            "node_bias": nb,
            "attention": att,
            "out_w": ow,
            "out_b": ob,
        })
    res = run_bass_kernel_spmd(nc, in_maps, core_ids=list(range(NCORES)))
    out = np.concatenate([res.results[i]["out"] for i in range(NCORES)], axis=0)
    return out.astype(np.float32)


# revision 24
# speedup vs baseline: 1.3504x; 1.3504x over previous
"""Trainium2 Bass kernel for nn_ADNN_57501022159565 (GNN message passing).

Reference computation (B=4096, D=256, N=128, F=128, O=256):
    conn = softmax(attention, axis=1)                      # [N, N]
    W_s  = node_weight[:, :F, :]                           # [N, F, F]
    W_x  = node_weight[:, F:, :]                           # [N, D, F]
    X    = einsum('bd,ndf->nbf', x, W_x) + bias[:,None,:]  # [N, B, F]
    S_1  = relu(X)                                         # (states0 = 0)
    S_k  = relu(einsum('ij,jbf->ibf', conn, S_{k-1}) @ W_s + X)
    out  = mean_n(S_T) @ out_w + out_b                     # [B, O]

Data-parallel over batch: B=4096 split across 8 NeuronCores (512 rows
each), weights replicated, no collectives. bf16 compute, f32 PSUM.

v3 design vs the 1.43ms v2 baseline:
  - The feature-major -> node-major rearrange of the state no longer
    issues 128 per-node HWDGE DMAs per chunk (the HWDGE ring was 59%
    busy, serializing the whole kernel). Instead the staging tiles are
    scattered into per-(iteration, chunk, half) DRAM scratch tensors in
    node-major layout (one HWDGE DMA per staging tile per half) and read
    back densely (one HWDGE DMA per half).
  - States are double-half-buffered ([N, F, 64] tiles, bufs=3) so the
    next chunk's state read overlaps the current chunk's aggregation.
  - Phase 1's node_bias rides the PSUM evictions (ACT Identity bias /
    DVE tensor_scalar_add with a per-partition bias column) instead of
    identity-matmul accumulation.
  - The iteration X-add still rides PSUM accumulation (identity MM).
  - Cross-chunk work (next chunk's S_1 emission, pair-1 phase 1) is
    emitted between k=2 and k=3 so it covers the rearrange gap.
"""
import sys

if "/opt/trn_rl_repo" not in sys.path:
    sys.path.insert(0, "/opt/trn_rl_repo")

import numpy as np

import concourse.bass as bass
import concourse.tile as tile
from concourse import bacc, mybir
from concourse.bass import AP
from concourse.bass_utils import run_bass_kernel_spmd
from concourse.masks import make_identity

F32 = mybir.dt.float32
BF16 = mybir.dt.bfloat16
AF = mybir.ActivationFunctionType


def _bcast(ap, n):
    """Append a stride-0 dim of size n to an AP (broadcast over batch)."""
    return AP(ap.tensor, ap.offset, list(ap.ap) + [[0, n]])

B, D, N, F, O = 4096, 256, 128, 128, 256
NCORES = 8
BLOC = B // NCORES          # 512 batch rows per core
BS = 128                    # batch chunk
HS = BS // 2                # state half-chunk
NCHUNK = BLOC // BS         # 4
NG = 16                     # node group for staging scatter-writes


def build_nc(num_iterations: int, reps: int = 1):
    T = int(num_iterations)
    nc = bacc.Bacc(None, target_bir_lowering=False)
    x_ext = nc.declare_dram_parameter("x", [BLOC, D], F32, isOutput=False)
    nw_ext = nc.declare_dram_parameter("node_weight", [N, F + D, F], F32, isOutput=False)
    nb_ext = nc.declare_dram_parameter("node_bias", [N, F], F32, isOutput=False)
    att_ext = nc.declare_dram_parameter("attention", [N, N], F32, isOutput=False)
    ow_ext = nc.declare_dram_parameter("out_w", [F, O], F32, isOutput=False)
    ob_ext = nc.declare_dram_parameter("out_b", [1, O], F32, isOutput=False)
    out_ext = nc.declare_dram_parameter("out", [BLOC, O], F32, isOutput=True)

    nw_r = nw_ext[:].rearrange("n d g -> d n g")  # (D+F, N, F) strided view

    # DRAM scratch for the state rearrange: one tensor per (rep, emission,
    # chunk) so there are no WAR/WAW hazards. Feature-major [F, N, BS]
    # (same layout as the staging tiles -> dense writes; the reads do the
    # node-major scatter, which HBM tolerates better than scattered
    # writes).
    n_emit = max(0, T - 1)
    sd = {}
    with tile.TileContext(nc) as tc:
        for r in range(reps):
            for k in range(n_emit):
                for c in range(NCHUNK):
                    sd[(r, k, c)] = nc.dram_tensor(
                        f"sd_{r}_{k}_{c}", [F, N, BS], BF16)
        with (
            tc.tile_pool(name="const", bufs=1) as cp,
            tc.tile_pool(name="wp", bufs=1) as wp,
            tc.tile_pool(name="big", bufs=1) as bp,
            tc.tile_pool(name="tmp", bufs=1) as tp,
            tc.tile_pool(name="stage", bufs=2) as sp,
            tc.tile_pool(name="wxp", bufs=2) as wxp,
            tc.tile_pool(name="ps_a", bufs=3, space="PSUM") as ps_a,
            tc.tile_pool(name="ps_m1", bufs=2, space="PSUM") as ps_m1,
            tc.tile_pool(name="ps_mI", bufs=2, space="PSUM") as ps_mI,
            tc.tile_pool(name="ps_o", bufs=1, space="PSUM") as ps_o,
        ):
            for _rep in range(reps):
                _body(nc, tc, T, cp, wp, bp, tp, sp, wxp, ps_a, ps_m1, ps_mI,
                      ps_o, x_ext, nw_r, nb_ext, att_ext, ow_ext, ob_ext,
                      out_ext,
                      {(k, c): sd[(_rep, k, c)]
                       for k in range(n_emit) for c in range(NCHUNK)})
    nc.compile()
    return nc


class _Alt:
    """Alternate eviction work between DVE and ACT."""

    def __init__(self, nc):
        self.nc = nc
        self.i = 0

    def copy(self, dst, src):
        self.i += 1
        if self.i % 2:
            self.nc.vector.tensor_copy(dst, src)
        else:
            self.nc.scalar.activation(dst, src, AF.Copy)

    def relu(self, dst, src):
        self.i += 1
        if self.i % 2:
            self.nc.scalar.activation(dst, src, AF.Relu)
        else:
            self.nc.vector.tensor_scalar_max(dst, src, 0.0)

    def add_bias(self, dst, src, bias_col):
        """dst = src + bias (bias: per-partition column AP)."""
        self.i += 1
        if self.i % 2:
            self.nc.scalar.activation(dst, src, AF.Identity, bias=bias_col)
        else:
            self.nc.vector.tensor_scalar_add(dst, src, bias_col)


def _body(nc, tc, T, cp, wp, bp, tp, sp, wxp, ps_a, ps_m1, ps_mI, ps_o,
          x_ext, nw_r, nb_ext, att_ext, ow_ext, ob_ext, out_ext, sd):
    alt = _Alt(nc)
    # ---------------- setup: constants ----------------
    ident = cp.tile([128, 128], BF16, tag="ident")
    make_identity(nc, ident[:])
    identf = cp.tile([128, 128], F32, tag="identf")
    make_identity(nc, identf[:])
    ones = cp.tile([1, 128], BF16, tag="ones")
    nc.gpsimd.memset(ones[:], 1.0)

    # x -> xT (d | b) bf16, two d-halves (f32 HWDGE load, PE transpose)
    xT = [cp.tile([128, BLOC], BF16, tag=f"xT{dc}", name=f"xT{dc}") for dc in range(2)]
    for c in range(NCHUNK):
        xs = sp.tile([BS, D], F32, tag="xs", bufs=2, name="xs")
        nc.sync.dma_start(xs[:], x_ext[c * BS:(c + 1) * BS, :])
        for dc in range(2):
            px = ps_o.tile([128, BS], F32, tag="poo")
            nc.tensor.transpose(px[:], xs[:, dc * 128:(dc + 1) * 128], identf[:])
            nc.vector.tensor_copy(xT[dc][:, c * BS:(c + 1) * BS], px[:])

    # out_w scaled by 1/N (folds the node-mean), out_b
    ow = cp.tile([F, O], BF16, tag="ow")
    nc.gpsimd.dma_start(ow[:], ow_ext[:])  # cast f32->bf16
    nc.vector.tensor_scalar_mul(ow[:], ow[:], 1.0 / N)
    ob = cp.tile([1, O], BF16, tag="ob")
    nc.gpsimd.dma_start(ob[:], ob_ext[:])  # cast f32->bf16

    if T == 0:
        po = ps_o.tile([BS, O], F32, tag="poo")
        nc.tensor.matmul(po[:], ones[:, 0:BS], ob[:], start=True, stop=True)
        ot = sp.tile([BS, O], F32, tag="ot", bufs=2, name="ot")
        nc.vector.tensor_copy(ot[:], po[:])
        for c in range(NCHUNK):
            nc.sync.dma_start(out_ext[c * BS:(c + 1) * BS, :], ot[:])
        return

    # softmax(attention) -> conn (bf16), then connT via PE transpose
    att = tp.tile([N, N], F32, tag="att")
    nc.sync.dma_start(att[:], att_ext[:])
    mx = tp.tile([N, 1], F32, tag="mx")
    nc.vector.tensor_reduce(mx[:], att[:], axis=mybir.AxisListType.X,
                            op=mybir.AluOpType.max, negate=True)
    ex = tp.tile([N, N], F32, tag="ex")
    nc.scalar.activation(ex[:], att[:], AF.Exp, bias=mx[:, 0:1])
    sm = tp.tile([N, 1], F32, tag="sm")
    nc.vector.tensor_reduce(sm[:], ex[:], axis=mybir.AxisListType.X,
                            op=mybir.AluOpType.add)
    rc = tp.tile([N, 1], F32, tag="rc")
    nc.vector.reciprocal(rc[:], sm[:])
    conn = tp.tile([N, N], BF16, tag="conn")
    nc.vector.tensor_scalar_mul(conn[:], ex[:], rc[:, 0:1])
    pt = ps_o.tile([N, N], BF16, tag="poo")
    nc.tensor.transpose(pt[:], conn[:], ident[:])
    connT = cp.tile([N, N], BF16, tag="connT")
    nc.vector.tensor_copy(connT[:], pt[:])

    # node_bias -> bias_fm (f | n) bf16 (folded via identity-MM into PSUM)
    nb_bf = tp.tile([N, F], BF16, tag="nb_bf")
    nc.gpsimd.dma_start(nb_bf[:], nb_ext[:])
    pb = ps_o.tile([F, N], BF16, tag="poo")
    nc.tensor.transpose(pb[:], nb_bf[:], ident[:])
    bias_fm = cp.tile([F, N], BF16, tag="bias_fm")
    nc.vector.tensor_copy(bias_fm[:], pb[:])

    # W_s resident (f | n, g) bf16 via SWDGE cast-DMA
    W_s = wp.tile([F, N, F], BF16, tag="W_s")

    X_tiles = {}     # chunk -> X tile

    NGX = 8          # node group for streamed W_x tiles

    def phase1_pair(p):
        """xW for chunks (2p, 2p+1): free dim 256, W_x streamed through a
        small rotating pool (SWDGE cast-DMA f32->bf16 on the Pool engine).
        node_bias is folded into the evictions (per-partition bias col)."""
        c0 = 2 * p
        cb = c0 * BS
        for c in (c0, c0 + 1):
            X_tiles[c] = bp.tile([F, N, BS], BF16, tag="X", bufs=2, name=f"X{c}")
        for base in range(0, N, NGX):
            wx0 = wxp.tile([128, NGX, F], BF16, tag="wx0", name="wx0")
            wx1 = wxp.tile([128, NGX, F], BF16, tag="wx1", name="wx1")
            nc.gpsimd.dma_start(wx0[:], nw_r[F:F + 128, base:base + NGX, :])
            nc.gpsimd.dma_start(wx1[:], nw_r[F + 128:F + 256, base:base + NGX, :])
            for n2 in range(0, NGX, 2):
                n0 = base + n2
                pm = ps_m1.tile([F, 2, 256], F32, tag="pm1")
                # one PSUM group per bank: bias broadcast writes first
                # (start=True overwrites), then the W_x matmuls accumulate
                nc.tensor.matmul(pm[:], ident[:], _bcast(bias_fm[:, n0:n0 + 2], 256),
                                 start=True, stop=False)
                for q in range(2):
                    nc.tensor.matmul(pm[:, q, :], wx0[:, n2 + q, :],
                                     xT[0][:, cb:cb + 256], start=False,
                                     stop=False)
                    nc.tensor.matmul(pm[:, q, :], wx1[:, n2 + q, :],
                                     xT[1][:, cb:cb + 256], start=False,
                                     stop=(q == 1))
                for ci in range(2):
                    alt.copy(X_tiles[c0 + ci][:, n0:n0 + 2, :],
                             pm[:, :, ci * BS:(ci + 1) * BS])

    def alloc_state(k, c):
        """Two batch-half state tiles [N, F, HS] from a 3-deep ring."""
        return [bp.tile([N, F, HS], BF16, tag="S", bufs=3,
                        name=f"S{k}_{c}_{h}") for h in range(2)]

    def emit_group(c, k, g0, gt, Sh):
        """One node group: dense write of the staging tile to DRAM, then
        two node-major scatter-reads (one per batch half) into Sh."""
        sdt = sd[(k, c)]
        nc.sync.dma_start(sdt[:, g0:g0 + NG, :], gt[:])
        for h in range(2):
            nc.sync.dma_start(
                Sh[h][g0:g0 + NG, :, :],
                sdt[:, g0:g0 + NG, h * HS:(h + 1) * HS].rearrange(
                    "f n b -> n f b"))

    def s1_emit(c):
        """S_1 = relu(X) -> staging -> DRAM -> node-major halves."""
        X = X_tiles[c]
        Sh = alloc_state(0, c)
        for g0 in range(0, N, NG):
            gt = sp.tile([F, NG, BS], BF16, tag="gt", bufs=3, name="gt")
            alt.relu(gt[:], X[:, g0:g0 + NG, :])
            emit_group(c, 0, g0, gt, Sh)
        return Sh

    def stationary(Sh, b):
        t = Sh[b // HS]
        return t[:, :, b % HS]

    def iteration(c, k, Sh):
        """One iteration k for chunk c: aggregation + per-node transform.
        Returns the next state half-tiles, or None for the last iteration
        (relu written into X in place)."""
        last = (k == T)
        X = X_tiles[c]
        agg = bp.tile([F, BS, N], BF16, tag="agg", name="agg")
        for b0 in range(0, BS, 4):
            pa = ps_a.tile([F, 4, N], F32, tag="pa")
            for q in range(4):
                nc.tensor.matmul(pa[:, q, :], stationary(Sh, b0 + q), connT[:],
                                 start=True, stop=True)
            alt.copy(agg[:, b0:b0 + 4, :], pa[:])
        S_next = None if last else alloc_state(k - 1, c)
        gt = None
        for i0 in range(0, N, 4):
            pm = ps_mI.tile([F, 4, BS], F32, tag="pmI")
            # X written first (start=True overwrites the bank), then the
            # per-node transforms accumulate into their slices
            nc.tensor.matmul(pm[:], ident[:], X[:, i0:i0 + 4, :],
                             start=True, stop=False)
            for q in range(4):
                nc.tensor.matmul(pm[:, q, :], W_s[:, i0 + q, :],
                                 agg[:, :, i0 + q], start=False,
                                 stop=(q == 3))
            if last:
                alt.relu(X[:, i0:i0 + 4, :], pm[:])
            else:
                if i0 % NG == 0:
                    gt = sp.tile([F, NG, BS], BF16, tag="gt", bufs=3,
                                 name="gt")
                alt.relu(gt[:, (i0 % NG):(i0 % NG) + 4, :], pm[:])
                if i0 % NG == NG - 4:
                    emit_group(c, k - 1, i0 - NG + 4, gt, S_next)
        return S_next

    def s1_to_slast(c):
        """T==1: S_T = relu(X) in place."""
        X = X_tiles[c]
        nc.vector.tensor_scalar_max(X[:], X[:], 0.0)
        return X

    def mean_and_out(c, ST):
        """Node-mean of the final state ST [f, n, b] by in-place halving
        adds over the node dim, then a strided reduce + final linear."""
        cs = slice(c * BS, (c + 1) * BS)
        # level-1 halving into the (dead) agg slot so ST=X frees right away
        m64 = bp.tile([F, 64, BS], BF16, tag="agg", name="m64")
        nc.vector.tensor_add(m64[:], ST[:, 0:64, :], ST[:, 64:128, :])
        nc.vector.tensor_add(m64[:, 0:32, :], m64[:, 0:32, :], m64[:, 32:64, :])
        nc.vector.tensor_add(m64[:, 0:16, :], m64[:, 0:16, :], m64[:, 16:32, :])
        macc = sp.tile([F, BS], F32, tag="macc", bufs=1, name="macc")
        nc.vector.tensor_reduce(macc[:], m64[:, 0:16, :].rearrange("f n b -> f b n"),
                                axis=mybir.AxisListType.X, op=mybir.AluOpType.add)
        mean_bf = sp.tile([F, BS], BF16, tag="mean_bf", bufs=1)
        nc.vector.tensor_copy(mean_bf[:], macc[:])
        po = ps_o.tile([BS, O], F32, tag="poo")
        nc.tensor.matmul(po[:], mean_bf[:], ow[:], start=True, stop=False)
        nc.tensor.matmul(po[:], ones[:, 0:BS], ob[:], start=False, stop=True)
        ot = sp.tile([BS, O], F32, tag="ot", bufs=1, name="ot")
        nc.scalar.activation(ot[:], po[:], AF.Copy)
        nc.sync.dma_start(out_ext[cs, :], ot[:])

    # ---------------- main pipeline ----------------
    phase1_pair(0)
    # W_s loads (SWDGE, after pair-0's W_x on the Pool queue)
    for base in range(0, N, 8):
        nc.gpsimd.dma_start(W_s[:, base:base + 8, :], nw_r[0:F, base:base + 8, :])

    if T >= 2:
        S_next = s1_emit(0)
        for c in range(NCHUNK):
            Sh = S_next
            for k in range(2, T + 1):
                Sh_new = iteration(c, k, Sh)
                if k == 2:
                    # cover the k=2 -> k=3 rearrange gap with independent
                    # cross-chunk work
                    if c == 1:
                        phase1_pair(1)
                    if c + 1 < NCHUNK:
                        S_next = s1_emit(c + 1)
                Sh = Sh_new
            mean_and_out(c, X_tiles[c])
    else:
        for c in range(NCHUNK):
            if c == 2:
                phase1_pair(1)
            mean_and_out(c, s1_to_slast(c))


_NC_CACHE = {}


def _get_nc(T: int):
    if T not in _NC_CACHE:
        _NC_CACHE[T] = build_nc(T)
    return _NC_CACHE[T]


def kernel(**inputs) -> np.ndarray:
    x = np.ascontiguousarray(np.asarray(inputs["x"], dtype=np.float32))
    nw = np.ascontiguousarray(np.asarray(inputs["node_weight"], dtype=np.float32))
    nb = np.ascontiguousarray(np.asarray(inputs["node_bias"], dtype=np.float32))
    att = np.ascontiguousarray(np.asarray(inputs["attention"], dtype=np.float32))
    ow = np.ascontiguousarray(np.asarray(inputs["out_w"], dtype=np.float32))
    ob = np.ascontiguousarray(np.asarray(inputs["out_b"], dtype=np.float32)).reshape(1, O)
    T = int(np.asarray(inputs["num_iterations"]))

    nc = _get_nc(T)
    in_maps = []
    for core in range(NCORES):
        shard = x[core * BLOC:(core + 1) * BLOC]
        in_maps.append({
            "x": shard,
            "node_weight": nw,
            "node_bias": nb,
            "attention": att,
            "out_w": ow,
            "out_b": ob,
        })
    res = run_bass_kernel_spmd(nc, in_maps, core_ids=list(range(NCORES)))
    out = np.concatenate([res.results[i]["out"] for i in range(NCORES)], axis=0)
    return out.astype(np.float32)
